# revision 1
# baseline (speedup 1.0000x reference)
"""Trainium2 Bass kernel for nn_NoiseConv1d (channel-wise 6-bit quantize + 1x1 conv).

Math (per batch b, position n, with QL=63):
    cmax/cmin = max/min over channels c of x[b,:,n]
    s = (cmax-cmin)/QL ; q = round((x-cmin)/s)   (q in [0,63], exact in fp16)
    out[o,n] = s[n] * sum_c W[o,c] q[c,n] + cmin[n]*Wsum[o] + bias[o]

Kernel strategy (one batch per NeuronCore, 8 cores data-parallel), mode "petr":
    - load x [512c, 4096n] natural layout (fp32)
    - PE-transpose 128x128 blocks -> PSUM, ACT-evac to SBUF xT [128n, 512c]
      (psum recycles fast; SBUF source gives DVE 2x mode)
    - DVE tensor_scalar+accum: per-n max and -min over channels (2x mode)
    - ACT quant: y+1536 = r*x + (-cmin*r + 1536) per-partition scale/bias fma;
      the fp32->fp16 output conversion RNE-rounds to integer+1536 for free
      (fp16 ulp == 1 in [1024, 2048))
    - DVE tensor_scalar: qs = (q16 - 1536) * s -> fp16 (exact q, one rounding)
    - cmin hi/lo fp16 split -> tiny [128,2] PE transpose; offset correction
      becomes a K=2 extra matmul with lhsT rows [Wsum_hi-ish, Wsum]
    - PE-transpose qs back to [c, n] (fp16 transpose-mode, 1 cyc/row) + DVE
      evac. (The xbar DMA-transpose path was 1.6x slower end-to-end: ~3us
      per call and it serializes against copy DMAs.)
    - PE matmul fp16 (psum fp32): 4 k-tiles + K=2 offset rows per [o,512n]
      psum tile; ACT evac folds the bias add; DMA out.
    Measured ~92.5 us/core on trn2 (8 cores parallel => full batch B=8);
    rel err vs reference ~1.4e-3.
"""

import sys

sys.path.insert(0, "/opt/trn_rl_repo")

import numpy as np

B, C, COUT, N = 8, 512, 512, 4096
QL = 63.0
MAGIC = float(np.float32(12582912.0))  # 1.5 * 2**23 : fp32 RNE round-to-int magic
MAGIC16 = 1536.0  # 1.5 * 2**10 : fp16 RNE round-to-int magic (ulp=1 in [1024,2048))

KT = C // 128  # 4 c-tiles
MT = COUT // 128  # 4 o-tiles
NT = N // 128  # 32 n-tiles
NCH = N // 512  # 8 n-chunks (psum free dim 512)

_cache = {}


def _build_bass(loop_n=0, mode="full"):
    from contextlib import ExitStack

    import concourse.bass as bass
    import concourse.mybir as mybir
    from concourse import bacc
    from concourse.bass import ds, ts
    from concourse.masks import make_identity
    from concourse.tile import TileContext

    f32 = mybir.dt.float32
    f16 = mybir.dt.float16
    AX = mybir.AxisListType
    OP = mybir.AluOpType
    AF = mybir.ActivationFunctionType

    nc = bacc.Bacc(None, target_bir_lowering=False)
    xb = nc.dram_tensor("xb", [C, N], f32, kind="ExternalInput")
    wt = nc.dram_tensor("wt", [C, COUT], f16, kind="ExternalInput")  # W^T [c,o]
    wr = nc.dram_tensor("wr", [2, COUT], f16, kind="ExternalInput")  # [Whi,Whi]
    bv = nc.dram_tensor("bv", [128, MT], f32, kind="ExternalInput")  # bias packed
    out = nc.dram_tensor("out", [COUT, N], f32, kind="ExternalOutput")

    with ExitStack() as ctx:
        tc = ctx.enter_context(TileContext(nc))
        singles = ctx.enter_context(tc.tile_pool(name="singles", bufs=1))

        NQ = N // 4  # n-quarter size
        if "cf16" in mode:
            x_nat = [
                singles.tile([128, KT, NQ], f16, name=f"x{q}", tag=f"x{q}")
                for q in range(4)
            ]
        else:
            x_nat = [
                singles.tile([128, KT, NQ], f32, name=f"x{q}", tag=f"x{q}")
                for q in range(4)
            ]
        if "f16x" in mode:
            # fp16 copy of x (natural layout) + its xbar-transposed form
            x_b = singles.tile([128, KT, N], f16)
            # per quarter: xtb[q][n%128, k, t_local, c%128] = x[c, n]
            xtb = [
                singles.tile([128, KT, NT // 4, 128], f16, name=f"xtb{q}", tag=f"xtb{q}")
                for q in range(4)
            ]
        wt_sb = singles.tile([128, KT, COUT], f16)
        wr_sb = singles.tile([2, COUT], f16)
        bv_sb = singles.tile([128, MT], f32)
        ident = singles.tile([128, 128], f32)
        ident16 = singles.tile([128, 128], f16)

        make_identity(nc, ident)
        make_identity(nc, ident16)
        nc.sync.dma_start(out=wt_sb, in_=wt.rearrange("(k p) o -> p k o", p=128))
        nc.sync.dma_start(out=wr_sb, in_=wr[:, :])
        nc.sync.dma_start(out=bv_sb, in_=bv[:, :])
        xq = xb.rearrange("(k p) n -> p k n", p=128)

        ps_tr = ctx.enter_context(tc.tile_pool(name="ps_tr", bufs=3, space="PSUM"))
        ps_qt = ctx.enter_context(tc.tile_pool(name="ps_qt", bufs=2, space="PSUM"))
        ps_mm = ctx.enter_context(tc.tile_pool(name="ps_mm", bufs=3, space="PSUM"))
        stat = ctx.enter_context(tc.tile_pool(name="stat", bufs=12))
        qpool = ctx.enter_context(tc.tile_pool(name="qpool", bufs=6))
        jpool = ctx.enter_context(tc.tile_pool(name="jpool", bufs=4))
        opool = ctx.enter_context(tc.tile_pool(name="opool", bufs=4))
        qtpool = ctx.enter_context(tc.tile_pool(name="qtpool", bufs=3))

        def per_iter():
            for q in range(4):
                if "cf16" in mode:
                    # SWDGE cast-DMA: fp32 DRAM -> fp16 SBUF directly
                    nc.gpsimd.dma_start(out=x_nat[q], in_=xq[:, :, ds(q * NQ, NQ)])
                else:
                    nc.sync.dma_start(out=x_nat[q], in_=xq[:, :, ds(q * NQ, NQ)])
            if "f16x" in mode:
                for q in range(4):
                    nc.vector.tensor_copy(
                        out=x_b[:, :, ds(q * NQ, NQ)], in_=x_nat[q]
                    )
                    for k in range(KT):
                        nc.sync.dma_start_transpose(
                            out=xtb[q][:, k],
                            in_=x_b[:, k, ds(q * NQ, NQ)],
                        )
            if "dma" in mode:
                # DMA floor: just stream junk out at full volume
                for j in range(NCH):
                    for m in range(MT):
                        nc.sync.dma_start(
                            out=out[ts(m, 128), ds(512 * j, 512)],
                            in_=x_nat[0][:, 0, 0:512],
                        )
                return
            body_chunks()

        def body_chunks():
          for j in range(NCH):
            # qsT[c%128, i, c//128, n%128]
            qsT = qtpool.tile([128, 4, KT, 128], f16, tag="qsT")
            # cmin hi/lo rows for this chunk: cmr[row, i, n%128]
            cmr = qtpool.tile([2, 4, 128], f16, tag="cmr")
            hl = stat.tile([128, 2, 4], f16, tag="hl")  # hi/lo per tile col
            for i in range(4):
                t = 4 * j + i
                if "f16x" in mode:
                    # x already transposed to [n, c] fp16 by the xbar DMA
                    xts = xtb[t // 8][:, :, t % 8, :]
                else:
                    # ---- transpose x block column t into [n, c] psum tile ----
                    cf = "cf16" in mode
                    xt_ps = ps_tr.tile([128, 512], f16 if cf else f32, tag="xt")
                    xs = x_nat[t // 8]
                    tl = t % 8
                    for k in range(KT):
                        nc.tensor.transpose(
                            xt_ps[:, ts(k, 128)],
                            xs[:, k, ts(tl, 128)],
                            ident16 if cf else ident,
                        )
                    # evac psum -> sbuf right away so psum slots recycle fast
                    # and the DVE stats below run at higher perf mode
                    xts_t = qpool.tile([128, 512], f16 if cf else f32, tag="xts")
                    nc.scalar.copy(out=xts_t, in_=xt_ps)
                    xts = xts_t
                # ---- per-n stats over free dim via tensor_scalar accumulate ----
                mx = stat.tile([128, 1], f32, tag="mx")
                ng = stat.tile([128, 1], f32, tag="ng")  # -cmin
                junk = jpool.tile([128, 512], f16, tag="junk")
                nc.vector.tensor_scalar(
                    out=junk, in0=xts, scalar1=1.0, scalar2=None,
                    op0=OP.mult, op1=OP.max, accum_out=mx,
                )
                nc.vector.tensor_scalar(
                    out=junk, in0=xts, scalar1=-1.0, scalar2=None,
                    op0=OP.mult, op1=OP.max, accum_out=ng,
                )
                d = stat.tile([128, 1], f32, tag="d")
                s = stat.tile([128, 1], f32, tag="s")
                r = stat.tile([128, 1], f32, tag="r")
                Bt = stat.tile([128, 1], f32, tag="Bt")
                nc.vector.tensor_add(d, mx, ng)  # d = cmax - cmin
                nc.vector.tensor_scalar_mul(s, d, 1.0 / QL)  # s = scaling
                nc.vector.reciprocal(r, s)  # r = 1/s
                # B = -cmin*r + 1536 (fp16 magic; fp32 ulp@1536 keeps fraction)
                nc.vector.tensor_scalar(
                    out=Bt, in0=ng, scalar1=r, scalar2=MAGIC16,
                    op0=OP.mult, op1=OP.add,
                )
                if "noq" in mode:
                    continue
                # ---- quantize: y+1536 = r*x + B via ACT fma; the fp32->fp16
                # output conversion RNE-rounds to integer+1536 exactly ----
                qp = qpool.tile([128, 512], f16, tag="qp")
                nc.scalar.activation(
                    out=qp, in_=xts, func=AF.Identity, bias=Bt, scale=r
                )
                # ---- qs = (q16 - 1536) * s -> fp16 ----
                qs = qpool.tile([128, KT * 128], f16, tag="qs")
                nc.vector.tensor_scalar(
                    out=qs, in0=qp, scalar1=MAGIC16, scalar2=s,
                    op0=OP.subtract, op1=OP.mult,
                )
                # cmin hi/lo for this tile: hi = f16(cmin); lo ~= cmin - hi
                nc.gpsimd.tensor_scalar_mul(hl[:, 0, i : i + 1], ng, -1.0)
                nc.gpsimd.tensor_scalar(
                    out=hl[:, 1, i : i + 1], in0=hl[:, 0, i : i + 1],
                    scalar1=ng, scalar2=-1.0, op0=OP.add, op1=OP.mult,
                )
                # ---- transpose back to [c, n] layout ----
                if "petr" in mode:
                    # PE transpose (f16, 1 cyc/row) + DVE evac; avoids the
                    # xbar DMA-transpose path entirely (slow + serializes
                    # against copy DMAs)
                    qt_ps = ps_qt.tile([128, (KT + 1) * 128], f16, tag="qtp")
                    for b_ in range(KT):
                        nc.tensor.transpose(
                            qt_ps[:, ts(b_, 128)], qs[:, ts(b_, 128)], ident16
                        )
                    nc.tensor.transpose(
                        qt_ps[0:2, ts(KT, 128)], hl[:, :, i], ident16
                    )
                    nc.vector.tensor_copy(out=qsT[:, i], in_=qt_ps[:, 0 : KT * 128])
                    nc.vector.tensor_copy(
                        out=cmr[:, i], in_=qt_ps[0:2, ts(KT, 128)]
                    )
                else:
                    nc.sync.dma_start_transpose(out=qsT[:, i], in_=qs)

            # ---- matmul for this n-chunk ----
            if "nomm" in mode:
                continue
            for m in range(MT):
                ps = ps_mm.tile([128, 512], f32, tag="mm")
                for k in range(KT):
                    nc.tensor.matmul(
                        ps,
                        wt_sb[:, k, ts(m, 128)],
                        qsT[:, :, k, :],
                        start=(k == 0),
                        stop=False,
                    )
                nc.tensor.matmul(
                    ps,
                    wr_sb[:, ts(m, 128)],
                    cmr[:, :, :],
                    start=False,
                    stop=True,
                )
                ob = opool.tile([128, 512], f32, tag="ob")
                nc.scalar.activation(
                    out=ob, in_=ps, func=AF.Identity,
                    bias=bv_sb[:, m : m + 1], scale=1.0,
                )
                nc.sync.dma_start(
                    out=out[ts(m, 128), ds(512 * j, 512)], in_=ob
                )

        if loop_n:
            with tc.For_i(0, loop_n, 1):
                per_iter()
        else:
            per_iter()

    nc.compile()
    return nc


def _prep_weights(weight, bias):
    W = weight[:, :, 0].astype(np.float64)  # [o, c]
    wt = np.ascontiguousarray(W.T).astype(np.float16)  # [c, o]
    wsum = W.sum(axis=1)  # [o]
    whi = wsum.astype(np.float16)
    wlo = (wsum - whi.astype(np.float64)).astype(np.float16)
    wr = np.stack([whi, whi], axis=0)  # [2, o] (rows pair with cmin hi/lo)
    bv = np.ascontiguousarray(bias.reshape(MT, 128).T).astype(np.float32)  # [128, MT]
    return wt, wr, bv


def kernel(x, weight, bias):
    from concourse.bass_utils import run_bass_kernel_spmd

    if "nc" not in _cache:
        _cache["nc"] = _build_bass(mode="petr")
    nc = _cache["nc"]

    wt, wr, bv = _prep_weights(np.asarray(weight), np.asarray(bias))
    x = np.asarray(x, dtype=np.float32)
    in_maps = [
        {
            "xb": np.ascontiguousarray(x[i]),
            "wt": wt,
            "wr": wr,
            "bv": bv,
        }
        for i in range(B)
    ]
    res = run_bass_kernel_spmd(nc, in_maps, core_ids=list(range(B)))
    return np.stack([r["out"] for r in res.results], axis=0).astype(np.float32)

